# revision 17
# baseline (speedup 1.0000x reference)
"""Trainium2 Bass kernel: batch-independent contrastive loss (SupCon-style with
EMA-normalized negatives).

Math (derived from the reference):
  CF = concat(views) [N=4096, D=256], S = CF @ CF.T / T, s_ij = f_i.f_j/T
  Each row i has exactly one positive p(i) = (i+B) mod N; neg_mask keeps the
  diagonal.  With m_i = ||f_i||^2/T:
    P_i  = sum_j exp(s_ij),  Q_i = sum_j exp(s_ij) s_ij
    Zneg_i = e^{-m_i} P_i - e_pos_i
    Wneg_i = e^{-m_i} (Q_i - m_i P_i) - e_pos_i Lpos_i
    u_new  = (1-g) u[idx] + g Zneg   (view-0 rows)
    loss_i = Wneg_i / u_new_{i mod B} - Lpos_i ;  output = mean_i loss_i

Key structural trick: with E'_ji = exp(s_ji - C + K) (C = 1/T ~ the common
row max since features are unit-norm, K = 8 a range shift), both P and Q are
matmul reductions over j:
    G_i[c]  = sum_j E'_ji f_j[c]   (c < 256)     -> Q_i = e^{C-K}/T f_i.G_i
    G_i[256]= sum_j E'_ji          (ones column) -> P_i = e^{C-K} G_i[256]
so the [N,N]-sized reduction work runs on the Tensor engine.

Per core (512 anchor rows i, all 4096 contrast j): 32 fp8 DoubleRow matmuls
build S^T in [128 j, 1024(=2x512 i)] PSUM pair-tiles; the Scalar engine does
exp (constant bias, no accumulate) into fp8-e5m2 E' tiles; 64 fp8 DoubleRow
G-matmuls accumulate G into 4 persistent PSUM banks; the Vector engine only
does tiny per-row stats, 4 [128,256] rowdots, and the final combine.  1/T is
folded into the anchor fp8 values on the host so PSUM holds s directly.
"""

import numpy as np
import ml_dtypes

GAMMA = 0.9
TEMP = 0.07
CB = 1.0 / TEMP      # common shift (row max for unit-norm features)
KS = 8.0             # range shift so e5m2 holds the mass
B, V, D = 2048, 2, 256
N = B * V            # 4096 contrast rows/cols
NCORES = 8
SPC = B // NCORES    # 256 samples per core
RPC = V * SPC        # 512 anchor rows per core
RC = RPC // 128      # 4 chunks of 128 anchor rows (0,1: view0; 2,3: view1)
NJC = N // 128       # 32 contrast chunks of 128
NJP = NJC // 2       # 16 chunk pairs
GW = 260             # G width: 256 features + ones col + pad

_CACHE = {}


def _build_module():
    import concourse.bacc as bacc
    import concourse.tile as tile
    from concourse import mybir

    f32 = mybir.dt.float32
    bf16 = mybir.dt.bfloat16
    f8e4 = mybir.dt.float8e4
    f8e5 = mybir.dt.float8e5
    AF = mybir.ActivationFunctionType
    ALU = mybir.AluOpType
    DR = mybir.MatmulPerfMode.DoubleRow

    nc = bacc.Bacc(
        "TRN2", target_bir_lowering=False, debug=False, enable_asserts=False
    )
    anc8_d = nc.dram_tensor("anc8", [128, 2 * RPC], f8e4, kind="ExternalInput")
    # ct quarters: [128 d-low, k-major x 1024 cols]; quarter q covers contrast
    # rows [q*1024, (q+1)*1024)
    ct8_d = nc.dram_tensor("ct8", [4, 128, 2 * 1024], f8e4, kind="ExternalInput")
    # F (G-matmul rhs): [128, jp-major x (2 halves x 260)] fp8, two DMA halves
    f8_d = nc.dram_tensor("f8", [128, NJP * 2 * GW], f8e4, kind="ExternalInput")
    fa_d = nc.dram_tensor("fa", [128, RC * D], bf16, kind="ExternalInput")
    ug_d = nc.dram_tensor("ug", [128, 2], f32, kind="ExternalInput")
    out_d = nc.dram_tensor("loss_rows", [128, RC], f32, kind="ExternalOutput")

    with tile.TileContext(nc) as tc:
        with tc.tile_pool(name="singles", bufs=1) as singles, \
             tc.tile_pool(name="spsum", bufs=2, space="PSUM") as spsum, \
             tc.tile_pool(name="gpsum", bufs=1, space="PSUM") as gpsum, \
             tc.tile_pool(name="epool", bufs=6) as epool, \
             tc.tile_pool(name="work", bufs=2) as work, \
             tc.tile_pool(name="stats", bufs=1) as stats:
            # ---- DMA (2 HWDGE rings: sync + scalar; no gpsimd SWDGE so the
            # exit drain stays cheap).  ct q0 split so the first matmul can
            # start as soon as a small first bite lands.
            anc8_flat = singles.tile([128, 2 * RPC], f8e4)
            nc.sync.dma_start(out=anc8_flat, in_=anc8_d[:, :])
            # q0 lives as two separate tiles so the first pair's matmuls gate
            # only on a small 64KB first bite
            ct_q0a = singles.tile([128, 2 * 256], f8e4)
            nc.sync.dma_start(out=ct_q0a[:, 0:256], in_=ct8_d[0][:, 0:256])
            nc.sync.dma_start(out=ct_q0a[:, 256:512],
                              in_=ct8_d[0][:, 1024:1024 + 256])
            ct_q0b = singles.tile([128, 2 * 768], f8e4)
            nc.sync.dma_start(out=ct_q0b[:, 0:768], in_=ct8_d[0][:, 256:1024])
            nc.sync.dma_start(out=ct_q0b[:, 768:1536],
                              in_=ct8_d[0][:, 1024 + 256:2048])
            ct_q = [None] * 4
            for q in range(1, 4):
                ct_tile = singles.tile([128, 2 * 1024], f8e4, tag=f"ct_{q}",
                                       name=f"ct_{q}")
                ct_q[q] = ct_tile
            # q1 on the gpsimd SWDGE ring (single transfer keeps its exit
            # drain small); F on the scalar ring; the rest trails on sync.
            nc.gpsimd.dma_start(out=ct_q[1], in_=ct8_d[1])
            f8_all = singles.tile([128, NJP * 2 * GW], f8e4)
            HF = NJP * GW  # half of the flat F payload
            nc.scalar.dma_start(out=f8_all[:, 0:HF], in_=f8_d[:, 0:HF])
            nc.scalar.dma_start(out=f8_all[:, HF:2 * HF], in_=f8_d[:, HF:2 * HF])
            for q in range(2, 4):
                nc.sync.dma_start(out=ct_q[q], in_=ct8_d[q])
            fa_flat = singles.tile([128, RC * D], bf16)
            nc.sync.dma_start(out=fa_flat, in_=fa_d[:, :])
            ug_sb = singles.tile([128, 2], f32)
            nc.sync.dma_start(out=ug_sb, in_=ug_d[:, :])

            anc_v = anc8_flat.rearrange("p (k r) -> p k r", k=2)
            fa_sb = fa_flat.rearrange("p (rc d) -> p rc d", rc=RC)
            ct_q0a_v = ct_q0a.rearrange("p (k j) -> p k j", k=2)
            ct_q0b_v = ct_q0b.rearrange("p (k j) -> p k j", k=2)
            ct_v = [None] + [ct_q[q].rearrange("p (k j) -> p k j", k=2)
                             for q in range(1, 4)]

            def ct_chunk(jc):
                # lhsT [128, 2, 128] for contrast chunk jc
                if jc < 2:
                    return ct_q0a_v[:, :, jc * 128:(jc + 1) * 128]
                if jc < 8:
                    return ct_q0b_v[:, :, (jc - 2) * 128:(jc - 1) * 128]
                q, r = jc // 8, jc % 8
                return ct_v[q][:, :, r * 128:(r + 1) * 128]
            f8_v = f8_all.rearrange("p (jp h c) -> p jp h c", jp=NJP, h=2)

            # persistent G accumulators (4 x 1 PSUM bank)
            gps = []
            for ic in range(4):
                g_acc = gpsum.tile([128, GW], f32, tag=f"g{ic}", name=f"g{ic}")
                gps.append(g_acc)

            # ---- per-row statistics from the bf16 anchor features ----
            msum = stats.tile([128, RC], f32)   # ||f_r||^2
            for rc in range(RC):
                scr2 = work.tile([128, D], f32, tag="scr2")
                nc.vector.scalar_tensor_tensor(
                    out=scr2, in0=fa_sb[:, rc, :], scalar=1.0,
                    in1=fa_sb[:, rc, :], op0=ALU.mult, op1=ALU.mult,
                    accum_out=msum[:, rc:rc + 1],
                )
            pd = stats.tile([128, 2], f32)      # f_view0 . f_view1 per sample
            for s in range(2):
                scr2 = work.tile([128, D], f32, tag="scr2")
                nc.vector.scalar_tensor_tensor(
                    out=scr2, in0=fa_sb[:, s, :], scalar=1.0,
                    in1=fa_sb[:, 2 + s, :], op0=ALU.mult, op1=ALU.mult,
                    accum_out=pd[:, s:s + 1],
                )
            bias_p = stats.tile([128, 1], f32)  # +(C-K) for em2
            nc.vector.memset(bias_p, CB - KS)
            bias_n = stats.tile([128, 1], f32)  # -(C-K) for the main exp
            nc.vector.memset(bias_n, KS - CB)
            m4 = stats.tile([128, RC], f32)     # m = msum/T
            nc.vector.tensor_scalar_mul(m4, msum, 1.0 / TEMP)
            em2 = stats.tile([128, RC], f32)    # e^{(C-K) - m}
            nc.scalar.activation(out=em2, in_=msum, func=AF.Exp,
                                 scale=-1.0 / TEMP, bias=bias_p)
            pd4 = stats.tile([128, RC], f32)
            nc.vector.tensor_copy(pd4[:, 0:2], pd)
            nc.vector.tensor_copy(pd4[:, 2:4], pd)
            lp2 = stats.tile([128, RC], f32)    # Lpos = pd/T - m
            nc.vector.scalar_tensor_tensor(
                out=lp2, in0=pd4, scalar=1.0 / TEMP, in1=m4,
                op0=ALU.mult, op1=ALU.subtract)
            ep = stats.tile([128, RC], f32)     # e_pos
            nc.scalar.activation(out=ep, in_=lp2, func=AF.Exp)
            epl = stats.tile([128, RC], f32)
            nc.vector.tensor_mul(epl, ep, lp2)

            # ---- main loop over 16 contrast chunk-pairs ----
            # PE warmup happens naturally: first S-matmuls gate on the anc/ct
            # DMAs; emit a few dummy DR matmuls first to ramp the clock.
            warm_sb = singles.tile([128, 2 * 256], f8e4)
            nc.vector.memset(warm_sb, 0.0)
            warm_v = warm_sb.rearrange("p (k j) -> p k j", k=2)

            e_tiles = [None] * NJP

            def emit_g(jp):
                e8v = e_tiles[jp].rearrange("p (h i) -> p h i", h=2)
                for ic in range(4):
                    nc.tensor.matmul(
                        gps[ic],
                        lhsT=e8v[:, :, ic * 128:(ic + 1) * 128],
                        rhs=f8_v[:, jp],
                        start=(jp == 0), stop=(jp == NJP - 1),
                        perf_mode=DR, skip_group_check=True,
                    )

            for jp in range(NJP):
                ps = spsum.tile([128, 1024], f32, tag="ps")
                if jp == 0:
                    for w in range(8):
                        nc.tensor.matmul(
                            ps[:, 0:256], lhsT=warm_v[:, :, 0:128],
                            rhs=warm_v, start=True, stop=True, perf_mode=DR)
                for h in range(2):
                    jc = 2 * jp + h
                    nc.tensor.matmul(
                        ps[:, h * 512:(h + 1) * 512],
                        lhsT=ct_chunk(jc),
                        rhs=anc_v,
                        start=True, stop=True, perf_mode=DR,
                    )
                e8 = epool.tile([128, 1024], f8e5, tag="e8")
                nc.scalar.activation(out=e8, in_=ps, func=AF.Exp,
                                     bias=bias_n)
                e_tiles[jp] = e8
                if jp >= 1:
                    emit_g(jp - 1)
            # final pair's G-matmuls interleaved with their rowdots so the
            # vector tail overlaps the last PE work
            qd4 = stats.tile([128, RC], f32)
            pacc4 = stats.tile([128, RC], f32)
            e8v_l = e_tiles[NJP - 1].rearrange("p (h i) -> p h i", h=2)
            for ic in range(4):
                nc.tensor.matmul(
                    gps[ic],
                    lhsT=e8v_l[:, :, ic * 128:(ic + 1) * 128],
                    rhs=f8_v[:, NJP - 1],
                    start=False, stop=True,
                    perf_mode=DR, skip_group_check=True,
                )
                scr3 = work.tile([128, D], f32, tag="scr3")
                nc.vector.scalar_tensor_tensor(
                    out=scr3, in0=gps[ic][:, 0:D], scalar=1.0,
                    in1=fa_sb[:, ic, :], op0=ALU.mult, op1=ALU.mult,
                    accum_out=qd4[:, ic:ic + 1],
                )
                nc.vector.tensor_copy(pacc4[:, ic:ic + 1], gps[ic][:, D:D + 1])

            mp4 = stats.tile([128, RC], f32)    # m * pacc
            nc.vector.tensor_mul(mp4, m4, pacc4)
            w4 = stats.tile([128, RC], f32)     # qd4/T - m*pacc
            nc.vector.scalar_tensor_tensor(
                out=w4, in0=qd4, scalar=1.0 / TEMP, in1=mp4,
                op0=ALU.mult, op1=ALU.subtract)
            wem = stats.tile([128, RC], f32)
            nc.vector.tensor_mul(wem, em2, w4)
            wn = stats.tile([128, RC], f32)     # Wneg
            nc.vector.tensor_sub(wn, wem, epl)

            z2 = stats.tile([128, 2], f32)
            nc.vector.tensor_mul(z2, em2[:, 0:2], pacc4[:, 0:2])
            zn2 = stats.tile([128, 2], f32)
            nc.vector.tensor_sub(zn2, z2, ep[:, 0:2])
            un = stats.tile([128, 2], f32)
            nc.vector.scalar_tensor_tensor(
                out=un, in0=zn2, scalar=GAMMA, in1=ug_sb,
                op0=ALU.mult, op1=ALU.add)
            ru = stats.tile([128, 2], f32)
            nc.vector.reciprocal(ru, un)
            c4 = stats.tile([128, RC], f32)
            nc.vector.tensor_mul(c4[:, 0:2], wn[:, 0:2], ru)
            nc.vector.tensor_mul(c4[:, 2:4], wn[:, 2:4], ru)
            out_sb = stats.tile([128, RC], f32)
            nc.vector.tensor_sub(out_sb, c4, lp2)
            nc.sync.dma_start(out=out_d[:, :], in_=out_sb)

    nc.compile()
    return nc


def _get_module():
    if "nc" not in _CACHE:
        _CACHE["nc"] = _build_module()
    return _CACHE["nc"]


def _prep_inputs(index, features, u):
    feats = np.asarray(features, dtype=np.float32)
    idx = np.asarray(index).astype(np.int64).reshape(-1)
    u_np = np.asarray(u, dtype=np.float32).reshape(-1)

    cf = np.ascontiguousarray(feats.transpose(1, 0, 2).reshape(N, D))
    cfb = cf.astype(ml_dtypes.bfloat16)
    ct = np.ascontiguousarray(cf.T)                        # [D, N] f32
    ct8 = ct.astype(ml_dtypes.float8_e4m3)
    # ct quarters [4, 128, k-major 1024]
    ct_in = np.ascontiguousarray(
        ct8.reshape(2, 128, 4, 1024).transpose(2, 1, 0, 3)
        .reshape(4, 128, 2 * 1024))
    # F for the G-matmul: [128 j-low, jp, h, 260]
    f8 = np.zeros((128, NJP, 2, GW), dtype=ml_dtypes.float8_e4m3)
    cf8 = cf.astype(ml_dtypes.float8_e4m3)                 # [N, D]
    f8[:, :, :, 0:D] = cf8.reshape(NJP, 2, 128, D).transpose(2, 0, 1, 3)
    f8[:, :, :, D] = np.float32(1.0)
    f8_in = np.ascontiguousarray(f8.reshape(128, NJP * 2 * GW))

    in_maps = []
    for c in range(NCORES):
        rows = np.concatenate([
            np.arange(c * SPC, (c + 1) * SPC),
            np.arange(B + c * SPC, B + (c + 1) * SPC),
        ])
        anc_r = (ct[:, rows] / TEMP).astype(ml_dtypes.float8_e4m3)
        anc = np.empty((128, 2 * RPC), dtype=ml_dtypes.float8_e4m3)
        anc[:, 0:RPC] = anc_r[0:128]
        anc[:, RPC:2 * RPC] = anc_r[128:256]
        fa_r = cfb[rows, :]                                # [RPC, D]
        fa = np.empty((128, RC * D), dtype=ml_dtypes.bfloat16)
        for rc in range(RC):
            fa[:, rc * D:(rc + 1) * D] = fa_r[rc * 128:(rc + 1) * 128]
        ug_vals = (1.0 - GAMMA) * u_np[idx[c * SPC:(c + 1) * SPC]]
        ug = np.ascontiguousarray(ug_vals.reshape(2, 128).T)  # [128, 2]
        in_maps.append({"anc8": anc, "fa": fa, "ug": ug, "ct8": ct_in,
                        "f8": f8_in})
    return in_maps


def _run(in_maps, trace=False, **kw):
    from concourse.bass_utils import run_bass_kernel_spmd

    nc = _get_module()
    return run_bass_kernel_spmd(
        nc, in_maps, core_ids=list(range(NCORES)), trace=trace, **kw
    )


def kernel(index, features, u):
    in_maps = _prep_inputs(index, features, u)
    res = _run(in_maps)
    total = 0.0
    for c in range(NCORES):
        total += np.asarray(res.results[c]["loss_rows"], dtype=np.float64).sum()
    return np.float32(total / N)
